# revision 13
# baseline (speedup 1.0000x reference)
"""Trainium2 Bass kernel for nn_Classifier_sep_model.

Reference computation (B=128, S=2048, H=768, L=26):
    sep_ids = sum(input_mask, axis=1)                        # [B]
    sep_outputs = hidden_output[b, sep_ids[b], :]            # [B, H] gather
    outs = concat([sep_outputs, cls_outputs], axis=1)        # [B, 2H]
    pred = outs @ W.T + b                                    # [B, L]

Sharding: data-parallel over B across 8 cores (16 samples/core); W, b
replicated.  On each core the kernel reads only the mask (128 KB) and the
16 needed rows of hidden_output via an indirect (gathered) DMA — it never
streams the 100 MB hidden slice.

Constraint worked around here: a PE Matmult lowers to LdWeights+Matmult
and the LdWeights struct accepts a single sync-wait, so no PE instruction
may introduce more than one not-yet-observed semaphore.  All PE-feeding
constants arrive in ONE packed DMA (one HWDGE sem lane), a warmup
transpose observes that lane first, and the remaining PE ops each add at
most one new semaphore.
"""

import numpy as np

import concourse.bass as bass
import concourse.tile as tile
from concourse import mybir
from concourse.bass_utils import run_bass_kernel_spmd
from concourse.vector_clock import ScopedClock, VectorClock


def _single_wait_drain_and_barrier(self, tick_clock, wait_clock):
    """Replacement for TileContext._drain_and_barrier emitting one
    single-wait Drain per outstanding proc: the walrus codegen used by the
    axon/PJRT path allows at most ONE sync-wait per instruction."""
    gclock = tick_clock.global_clock
    n = len(gclock)
    for proc in range(n):
        t = gclock[proc]
        if t > 0:
            partial = VectorClock([t if i == proc else 0 for i in range(n)])
            d = self.nc.sync.drain()
            wait_clock.add_sem_waits(d.ins, ScopedClock({None: partial}))
    self.nc.all_engine_barrier()
    assert self.sems is not None
    popped = self.nc._tile_sem_poison_stack.pop()
    assert popped is self._sem_poison
    self.nc.clear_and_free_semaphores(list(self.sems.allocated().values()))
    self.nc.all_engine_barrier()


tile.TileContext._drain_and_barrier = _single_wait_drain_and_barrier

B, S, H, L = 128, 2048, 768, 26
NCORES = 8
BC = B // NCORES          # 16 samples per core
KC = 2 * H // 128         # 12 contraction chunks of 128
SC = H // 128             # 6 chunks each for sep / cls halves
MQ = BC * S // 128        # 256 mask columns in [128, 256] layout

# packed constant buffer column layout (f32, 128 partitions)
C_WT = 0                  # [128, 312]  W^T chunks: wt[p, c*L + l] = W[l, c*128+p]
C_ONES = C_WT + KC * L    # [128, 16]   onesblk[p, j] = (p // 8 == j)
C_EYE = C_ONES + BC       # [:16, 16]   eye16 (partitions 16..127 zero)
C_BIAS = C_EYE + BC       # [:16, 26]   bias replicated to 16 rows
C_ROWOFF = C_BIAS + L     # [:1, 16]    row j -> j*S (flat gather base)
C_ONE1 = C_ROWOFF + BC    # [:1, 1]     1.0
C_COLS = C_ONE1 + 1       # 387

_PROG = None


def _build_program():
    nc = bass.Bass("TRN2", target_bir_lowering=False, debug=False,
                   num_devices=NCORES)
    f32, i32 = mybir.dt.float32, mybir.dt.int32

    hid = nc.dram_tensor("hidden", [BC * S, H], f32, kind="ExternalInput")
    clsd = nc.dram_tensor("cls", [BC, H], f32, kind="ExternalInput")
    maskd = nc.dram_tensor("mask", [128, MQ], i32, kind="ExternalInput")
    packd = nc.dram_tensor("cpack", [128, C_COLS], f32, kind="ExternalInput")
    outd = nc.dram_tensor("out", [BC, L], f32, kind="ExternalOutput")

    with tile.TileContext(nc) as tc:
        with tc.tile_pool(name="sb", bufs=1) as sb, \
             tc.tile_pool(name="ps1", bufs=1, space="PSUM") as ps1, \
             tc.tile_pool(name="ps2", bufs=3, space="PSUM") as ps2:
            # ---- input DMAs ----
            cpack = sb.tile([128, C_COLS], f32)
            nc.sync.dma_start(out=cpack[:], in_=packd.ap())
            cls_sb = sb.tile([BC, H], f32)
            nc.sync.dma_start(out=cls_sb[:], in_=clsd.ap())
            mask_t = sb.tile([128, MQ], i32)
            nc.sync.dma_start(out=mask_t[:], in_=maskd.ap())

            wT = cpack[:, C_WT:C_WT + KC * L]
            onesblk = cpack[:, C_ONES:C_ONES + BC]
            eye = cpack[:16, C_EYE:C_EYE + BC]
            bias_ap = cpack[:16, C_BIAS:C_BIAS + L]

            # PE warmup: observe the cpack DMA lane once so wT/onesblk/eye
            # are "seen" by every later PE instruction.
            trash = ps1.tile([BC, BC], f32)
            nc.tensor.transpose(out=trash[:], in_=eye, identity=eye)

            # bias to its own tile so the final DVE add doesn't need the
            # cpack semaphore
            bias_sb = sb.tile([BC, L], f32)
            nc.vector.tensor_copy(out=bias_sb[:], in_=bias_ap)

            # cls rows, transposed on PE into [K=128, b] chunks
            clsT = sb.tile([128, SC, BC], f32)
            for c in range(SC):
                tr = ps2.tile([128, BC], f32)
                nc.tensor.transpose(out=tr[:], in_=cls_sb[:, c * 128:(c + 1) * 128],
                                    identity=eye)
                nc.scalar.copy(out=clsT[:, c, :], in_=tr[:])

            # ---- mask -> sep_ids -> flat gather indices (critical path) ----
            sums_i = sb.tile([128, 1], i32)
            with nc.allow_low_precision(reason="int32 sum of 0/1 mask is exact"):
                nc.vector.tensor_reduce(out=sums_i[:], in_=mask_t[:],
                                        axis=mybir.AxisListType.X,
                                        op=mybir.AluOpType.add)
            sums_f = sb.tile([128, 1], f32)
            nc.vector.tensor_copy(out=sums_f[:], in_=sums_i[:])
            # per-sample sums: group-of-8-partitions reduction via matmul,
            # plus a K=1 accumulation adding the per-row flat base j*S
            sep_psum = ps1.tile([BC, 1], f32)
            nc.tensor.matmul(out=sep_psum[:], lhsT=cpack[:1, C_ROWOFF:C_ROWOFF + BC],
                             rhs=cpack[:1, C_ONE1:C_ONE1 + 1],
                             start=True, stop=False)
            nc.tensor.matmul(out=sep_psum[:], lhsT=onesblk, rhs=sums_f[:],
                             start=False, stop=True)
            idx = sb.tile([BC, 1], i32)
            nc.vector.tensor_copy(out=idx[:], in_=sep_psum[:])

            # ---- gather the 16 sep rows straight from DRAM ----
            sep_rows = sb.tile([BC, H], f32)
            nc.gpsimd.indirect_dma_start(
                out=sep_rows[:], out_offset=None,
                in_=hid.ap(),
                in_offset=bass.IndirectOffsetOnAxis(ap=idx[:, :1], axis=0),
            )

            sepT = sb.tile([128, SC, BC], f32)
            for c in range(SC):
                tr = ps2.tile([128, BC], f32)
                nc.tensor.transpose(out=tr[:], in_=sep_rows[:, c * 128:(c + 1) * 128],
                                    identity=eye)
                nc.scalar.copy(out=sepT[:, c, :], in_=tr[:])

            # ---- pred = [sep | cls] @ W.T, K=1536 in 12 chunks ----
            pred = ps1.tile([BC, L], f32)
            for c in range(SC):
                nc.tensor.matmul(out=pred[:], lhsT=clsT[:, c, :],
                                 rhs=wT[:, (SC + c) * L:(SC + c + 1) * L],
                                 start=(c == 0), stop=False)
            for c in range(SC):
                nc.tensor.matmul(out=pred[:], lhsT=sepT[:, c, :],
                                 rhs=wT[:, c * L:(c + 1) * L], start=False,
                                 stop=(c == SC - 1))

            # ---- + bias, write out ----
            out_sb = sb.tile([BC, L], f32)
            nc.vector.tensor_tensor(out=out_sb[:], in0=pred[:], in1=bias_sb[:],
                                    op=mybir.AluOpType.add)
            nc.sync.dma_start(out=outd.ap(), in_=out_sb[:])
    return nc


def _get_program():
    global _PROG
    if _PROG is None:
        _PROG = _build_program()
    return _PROG


def _make_in_maps(hidden_output, cls_outputs, input_mask, W, b):
    cpack = np.zeros((128, C_COLS), dtype=np.float32)
    # W[l, k] with k = c*128 + p  ->  wt[p, c*26 + l]
    cpack[:, C_WT:C_WT + KC * L] = np.ascontiguousarray(
        W.reshape(L, KC, 128).transpose(2, 1, 0)).reshape(128, KC * L)
    cpack[:, C_ONES:C_ONES + BC] = np.repeat(
        np.eye(BC, dtype=np.float32), 128 // BC, axis=0)
    cpack[:BC, C_EYE:C_EYE + BC] = np.eye(BC, dtype=np.float32)
    cpack[:BC, C_BIAS:C_BIAS + L] = b.reshape(1, L)
    cpack[0, C_ROWOFF:C_ROWOFF + BC] = np.arange(BC, dtype=np.float32) * S
    cpack[0, C_ONE1] = 1.0

    in_maps = []
    for i in range(NCORES):
        s = slice(i * BC, (i + 1) * BC)
        in_maps.append({
            "hidden": np.ascontiguousarray(hidden_output[s]).reshape(BC * S, H),
            "cls": np.ascontiguousarray(cls_outputs[s]),
            "mask": np.ascontiguousarray(input_mask[s]).reshape(128, MQ),
            "cpack": cpack,
        })
    return in_maps


def kernel(hidden_output, cls_outputs, input_mask, W, b, **run_kwargs):
    nc = _get_program()
    in_maps = _make_in_maps(
        np.asarray(hidden_output, dtype=np.float32),
        np.asarray(cls_outputs, dtype=np.float32),
        np.asarray(input_mask, dtype=np.int32),
        np.asarray(W, dtype=np.float32),
        np.asarray(b, dtype=np.float32),
    )
    res = run_bass_kernel_spmd(nc, in_maps, core_ids=list(range(NCORES)),
                               **run_kwargs)
    out = np.concatenate([r["out"] for r in res.results], axis=0)
    if run_kwargs:
        return out, res
    return out


# revision 26
# speedup vs baseline: 1.0495x; 1.0495x over previous
"""Trainium2 Bass kernel for nn_Classifier_sep_model.

Reference computation (B=128, S=2048, H=768, L=26):
    sep_ids = sum(input_mask, axis=1)                        # [B]
    sep_outputs = hidden_output[b, sep_ids[b], :]            # [B, H] gather
    outs = concat([sep_outputs, cls_outputs], axis=1)        # [B, 2H]
    pred = outs @ W.T + b                                    # [B, L]

Sharding: data-parallel over B across 8 cores (16 samples/core); W, b
replicated.  On each core the kernel reads only the mask (128 KB) and the
16 needed rows of hidden_output via an indirect (gathered) DMA — it never
streams the 100 MB hidden slice.

Constraint worked around here: a PE Matmult lowers to LdWeights+Matmult
and the LdWeights struct accepts a single sync-wait, so no PE instruction
may introduce more than one not-yet-observed semaphore.  All PE-feeding
constants arrive in ONE packed DMA (one HWDGE sem lane), a warmup
transpose observes that lane first, and the remaining PE ops each add at
most one new semaphore.
"""

import numpy as np

import concourse.bass as bass
import concourse.tile as tile
from concourse import mybir
from concourse.bass_utils import run_bass_kernel_spmd
from concourse.vector_clock import ScopedClock, VectorClock


def _single_wait_drain_and_barrier(self, tick_clock, wait_clock):
    """Replacement for TileContext._drain_and_barrier emitting one
    single-wait Drain per outstanding proc: the walrus codegen used by the
    axon/PJRT path allows at most ONE sync-wait per instruction."""
    gclock = tick_clock.global_clock
    n = len(gclock)
    for proc in range(n):
        t = gclock[proc]
        if t > 0:
            partial = VectorClock([t if i == proc else 0 for i in range(n)])
            d = self.nc.sync.drain()
            wait_clock.add_sem_waits(d.ins, ScopedClock({None: partial}))
    self.nc.all_engine_barrier()
    assert self.sems is not None
    popped = self.nc._tile_sem_poison_stack.pop()
    assert popped is self._sem_poison
    self.nc.clear_and_free_semaphores(list(self.sems.allocated().values()))
    self.nc.all_engine_barrier()


tile.TileContext._drain_and_barrier = _single_wait_drain_and_barrier

B, S, H, L = 128, 2048, 768, 26
NCORES = 8
BC = B // NCORES          # 16 samples per core
KC = 2 * H // 128         # 12 contraction chunks of 128
SC = H // 128             # 6 chunks each for sep / cls halves
MQ = BC * S // 128        # 256 mask columns in [128, 256] layout

# packed constant buffer column layout (f32, 128 partitions)
C_WT = 0                  # [128, 312]  W^T chunks: wt[p, c*L + l] = W[l, c*128+p]
C_ONES = C_WT + KC * L    # [128, 16]   onesblk[p, j] = (p // 8 == j)
C_EYE = C_ONES + BC       # [:16, 16]   eye16 (partitions 16..127 zero)
C_ROWOFF = C_EYE + BC     # [:1, 16]    row j -> j*S (flat gather base)
C_ONE1 = C_ROWOFF + BC    # [:1, 1]     1.0
C_BIASR = C_ONE1 + 1      # [:1, 26]    bias row (partition 0)
C_ONER = C_BIASR + L      # [:1, 16]    ones row (partition 0)
C_COLS = C_ONER + BC      # 403

_PROG = None


def _build_program():
    nc = bass.Bass("TRN2", target_bir_lowering=False, debug=False,
                   num_devices=NCORES, enable_partition_id=False,
                   monotonic_sem_count=0)
    f32, i32 = mybir.dt.float32, mybir.dt.int32

    hid = nc.dram_tensor("hidden", [BC * S, H], f32, kind="ExternalInput")
    clsd = nc.dram_tensor("cls", [BC, H], f32, kind="ExternalInput")
    maskd = nc.dram_tensor("mask", [128, MQ], i32, kind="ExternalInput")
    packd = nc.dram_tensor("cpack", [128, C_COLS], f32, kind="ExternalInput")
    outd = nc.dram_tensor("out", [BC, L], f32, kind="ExternalOutput")

    with tile.TileContext(nc) as tc:
        with tc.tile_pool(name="sb", bufs=1) as sb, \
             tc.tile_pool(name="ps1", bufs=1, space="PSUM") as ps1, \
             tc.tile_pool(name="ps2", bufs=1, space="PSUM") as ps2:
            # ---- input DMAs: mask first (critical path), constants on the
            # ACT HWDGE ring so its descriptor-gen overlaps SP's ----
            mask_t = sb.tile([128, MQ], i32)
            nc.sync.dma_start(out=mask_t[:], in_=maskd.ap())
            cpack = sb.tile([128, C_COLS], f32)
            nc.scalar.dma_start(out=cpack[:], in_=packd.ap())
            cls_sb = sb.tile([BC, H], f32)
            nc.sync.dma_start(out=cls_sb[:], in_=clsd.ap())

            wT = cpack[:, C_WT:C_WT + KC * L]
            onesblk = cpack[:, C_ONES:C_ONES + BC]
            eye = cpack[:16, C_EYE:C_EYE + BC]

            # PE warmup: observe the cpack DMA lane once so wT/onesblk/eye
            # are "seen" by every later PE instruction.
            trash = ps1.tile([BC, BC], f32)
            nc.tensor.transpose(out=trash[:], in_=eye, identity=eye)

            # cls rows, transposed on PE into [K=128, b] chunks; all 6
            # transposes land in one PSUM bank -> single copy to SBUF
            clsT = sb.tile([128, SC, BC], f32)
            cls_ps = ps2.tile([128, SC, BC], f32)
            for c in range(SC):
                nc.tensor.transpose(out=cls_ps[:, c, :],
                                    in_=cls_sb[:, c * 128:(c + 1) * 128],
                                    identity=eye)
            nc.vector.tensor_copy(out=clsT[:], in_=cls_ps[:])

            # ---- mask -> sep_ids -> flat gather indices (critical path) ----
            sums_i = sb.tile([128, 1], i32)
            with nc.allow_low_precision(reason="int32 sum of 0/1 mask is exact"):
                nc.vector.tensor_reduce(out=sums_i[:], in_=mask_t[:],
                                        axis=mybir.AxisListType.X,
                                        op=mybir.AluOpType.add)
            sums_f = sb.tile([128, 1], f32)
            nc.vector.tensor_copy(out=sums_f[:], in_=sums_i[:])
            # per-sample sums: group-of-8-partitions reduction via matmul,
            # plus a K=1 accumulation adding the per-row flat base j*S
            sep_psum = ps1.tile([BC, 1], f32)
            nc.tensor.matmul(out=sep_psum[:], lhsT=cpack[:1, C_ROWOFF:C_ROWOFF + BC],
                             rhs=cpack[:1, C_ONE1:C_ONE1 + 1],
                             start=True, stop=False)
            nc.tensor.matmul(out=sep_psum[:], lhsT=onesblk, rhs=sums_f[:],
                             start=False, stop=True)
            idx = sb.tile([BC, 1], i32)
            nc.vector.tensor_copy(out=idx[:], in_=sep_psum[:])

            # ---- gather the 16 sep rows straight from DRAM ----
            sep_rows = sb.tile([BC, H], f32)
            nc.gpsimd.indirect_dma_start(
                out=sep_rows[:], out_offset=None,
                in_=hid.ap(),
                in_offset=bass.IndirectOffsetOnAxis(ap=idx[:, :1], axis=0),
            )

            sepT = sb.tile([128, SC, BC], f32)
            sep_ps = ps2.tile([128, SC, BC], f32)
            for c in range(SC):
                nc.tensor.transpose(out=sep_ps[:, c, :],
                                    in_=sep_rows[:, c * 128:(c + 1) * 128],
                                    identity=eye)
            nc.vector.tensor_copy(out=sepT[:], in_=sep_ps[:])

            # ---- pred = [sep | cls] @ W.T + b: bias as a K=1 matmul from
            # cpack (zero new sems on PE), then 12 K-chunks ----
            pred = ps1.tile([BC, L], f32)
            nc.tensor.matmul(out=pred[:], lhsT=cpack[:1, C_ONER:C_ONER + BC],
                             rhs=cpack[:1, C_BIASR:C_BIASR + L],
                             start=True, stop=False)
            for c in range(SC):
                nc.tensor.matmul(out=pred[:], lhsT=clsT[:, c, :],
                                 rhs=wT[:, (SC + c) * L:(SC + c + 1) * L],
                                 start=False, stop=False)
            for c in range(SC):
                nc.tensor.matmul(out=pred[:], lhsT=sepT[:, c, :],
                                 rhs=wT[:, c * L:(c + 1) * L], start=False,
                                 stop=(c == SC - 1))

            out_sb = sb.tile([BC, L], f32)
            nc.vector.tensor_copy(out=out_sb[:], in_=pred[:])
            nc.sync.dma_start(out=outd.ap(), in_=out_sb[:])
    return nc


def _get_program():
    global _PROG
    if _PROG is None:
        _PROG = _build_program()
    return _PROG


def _make_in_maps(hidden_output, cls_outputs, input_mask, W, b):
    cpack = np.zeros((128, C_COLS), dtype=np.float32)
    # W[l, k] with k = c*128 + p  ->  wt[p, c*26 + l]
    cpack[:, C_WT:C_WT + KC * L] = np.ascontiguousarray(
        W.reshape(L, KC, 128).transpose(2, 1, 0)).reshape(128, KC * L)
    cpack[:, C_ONES:C_ONES + BC] = np.repeat(
        np.eye(BC, dtype=np.float32), 128 // BC, axis=0)
    cpack[:BC, C_EYE:C_EYE + BC] = np.eye(BC, dtype=np.float32)
    cpack[0, C_ROWOFF:C_ROWOFF + BC] = np.arange(BC, dtype=np.float32) * S
    cpack[0, C_ONE1] = 1.0
    cpack[0, C_BIASR:C_BIASR + L] = b
    cpack[0, C_ONER:C_ONER + BC] = 1.0

    in_maps = []
    for i in range(NCORES):
        s = slice(i * BC, (i + 1) * BC)
        in_maps.append({
            "hidden": np.ascontiguousarray(hidden_output[s]).reshape(BC * S, H),
            "cls": np.ascontiguousarray(cls_outputs[s]),
            "mask": np.ascontiguousarray(input_mask[s]).reshape(128, MQ),
            "cpack": cpack,
        })
    return in_maps


def kernel(hidden_output, cls_outputs, input_mask, W, b, **run_kwargs):
    nc = _get_program()
    in_maps = _make_in_maps(
        np.asarray(hidden_output, dtype=np.float32),
        np.asarray(cls_outputs, dtype=np.float32),
        np.asarray(input_mask, dtype=np.int32),
        np.asarray(W, dtype=np.float32),
        np.asarray(b, dtype=np.float32),
    )
    res = run_bass_kernel_spmd(nc, in_maps, core_ids=list(range(NCORES)),
                               **run_kwargs)
    out = np.concatenate([r["out"] for r in res.results], axis=0)
    if run_kwargs:
        return out, res
    return out


# revision 28
# speedup vs baseline: 1.0723x; 1.0217x over previous
"""Trainium2 Bass kernel for nn_Classifier_sep_model.

Reference computation (B=128, S=2048, H=768, L=26):
    sep_ids = sum(input_mask, axis=1)                        # [B]
    sep_outputs = hidden_output[b, sep_ids[b], :]            # [B, H] gather
    outs = concat([sep_outputs, cls_outputs], axis=1)        # [B, 2H]
    pred = outs @ W.T + b                                    # [B, L]

Sharding: data-parallel over B across 8 cores (16 samples/core); W, b
replicated.  On each core the kernel reads only the mask (128 KB) and the
16 needed rows of hidden_output via an indirect (gathered) DMA — it never
streams the 100 MB hidden slice.

Constraint worked around here: a PE Matmult lowers to LdWeights+Matmult
and the LdWeights struct accepts a single sync-wait, so no PE instruction
may introduce more than one not-yet-observed semaphore.  All PE-feeding
constants arrive in ONE packed DMA (one HWDGE sem lane), a warmup
transpose observes that lane first, and the remaining PE ops each add at
most one new semaphore.
"""

import numpy as np

import concourse.bass as bass
import concourse.tile as tile
from concourse import mybir
from concourse.bass_utils import run_bass_kernel_spmd
from concourse.vector_clock import ScopedClock, VectorClock


def _single_wait_drain_and_barrier(self, tick_clock, wait_clock):
    """Replacement for TileContext._drain_and_barrier emitting one
    single-wait Drain per outstanding proc: the walrus codegen used by the
    axon/PJRT path allows at most ONE sync-wait per instruction."""
    gclock = tick_clock.global_clock
    n = len(gclock)
    for proc in range(n):
        t = gclock[proc]
        if t > 0:
            partial = VectorClock([t if i == proc else 0 for i in range(n)])
            d = self.nc.sync.drain()
            wait_clock.add_sem_waits(d.ins, ScopedClock({None: partial}))
    self.nc.all_engine_barrier()
    assert self.sems is not None
    popped = self.nc._tile_sem_poison_stack.pop()
    assert popped is self._sem_poison
    self.nc.clear_and_free_semaphores(list(self.sems.allocated().values()))
    # no trailing all_engine_barrier: the sem clear is gpsimd's final
    # instruction and the runtime end-of-program sync covers it


tile.TileContext._drain_and_barrier = _single_wait_drain_and_barrier

B, S, H, L = 128, 2048, 768, 26
NCORES = 8
BC = B // NCORES          # 16 samples per core
KC = 2 * H // 128         # 12 contraction chunks of 128
SC = H // 128             # 6 chunks each for sep / cls halves
MQ = BC * S // 128        # 256 mask columns in [128, 256] layout

# packed constant buffer column layout (f32, 128 partitions)
C_WT = 0                  # [128, 312]  W^T chunks: wt[p, c*L + l] = W[l, c*128+p]
C_ONES = C_WT + KC * L    # [128, 16]   onesblk[p, j] = (p // 8 == j)
C_EYE = C_ONES + BC       # [:16, 16]   eye16 (partitions 16..127 zero)
C_ROWOFF = C_EYE + BC     # [:1, 16]    row j -> j*S (flat gather base)
C_ONE1 = C_ROWOFF + BC    # [:1, 1]     1.0
C_BIASR = C_ONE1 + 1      # [:1, 26]    bias row (partition 0)
C_ONER = C_BIASR + L      # [:1, 16]    ones row (partition 0)
C_COLS = C_ONER + BC      # 403

_PROG = None


def _build_program():
    nc = bass.Bass("TRN2", target_bir_lowering=False, debug=False,
                   num_devices=1, enable_partition_id=False,
                   monotonic_sem_count=0)
    f32, i32 = mybir.dt.float32, mybir.dt.int32

    hid = nc.dram_tensor("hidden", [BC * S, H], f32, kind="ExternalInput")
    clsd = nc.dram_tensor("cls", [BC, H], f32, kind="ExternalInput")
    maskd = nc.dram_tensor("mask", [128, MQ], i32, kind="ExternalInput")
    packd = nc.dram_tensor("cpack", [128, C_COLS], f32, kind="ExternalInput")
    outd = nc.dram_tensor("out", [BC, L], f32, kind="ExternalOutput")

    with tile.TileContext(nc) as tc:
        with tc.tile_pool(name="sb", bufs=1) as sb, \
             tc.tile_pool(name="ps1", bufs=1, space="PSUM") as ps1, \
             tc.tile_pool(name="ps2", bufs=1, space="PSUM") as ps2:
            # ---- input DMAs: mask first (critical path), constants on the
            # ACT HWDGE ring so its descriptor-gen overlaps SP's ----
            mask_t = sb.tile([128, MQ], i32)
            nc.sync.dma_start(out=mask_t[:], in_=maskd.ap())
            cpack = sb.tile([128, C_COLS], f32)
            nc.scalar.dma_start(out=cpack[:], in_=packd.ap())
            cls_sb = sb.tile([BC, H], f32)
            nc.sync.dma_start(out=cls_sb[:], in_=clsd.ap())

            wT = cpack[:, C_WT:C_WT + KC * L]
            onesblk = cpack[:, C_ONES:C_ONES + BC]
            eye = cpack[:16, C_EYE:C_EYE + BC]

            # PE warmup: observe the cpack DMA lane once so wT/onesblk/eye
            # are "seen" by every later PE instruction.
            trash = ps1.tile([BC, BC], f32)
            nc.tensor.transpose(out=trash[:], in_=eye, identity=eye)

            # cls rows, transposed on PE into [K=128, b] chunks; all 6
            # transposes land in one PSUM bank -> single copy to SBUF
            clsT = sb.tile([128, SC, BC], f32)
            cls_ps = ps2.tile([128, SC, BC], f32)
            for c in range(SC):
                nc.tensor.transpose(out=cls_ps[:, c, :],
                                    in_=cls_sb[:, c * 128:(c + 1) * 128],
                                    identity=eye)
            nc.vector.tensor_copy(out=clsT[:], in_=cls_ps[:])

            # ---- mask -> sep_ids -> flat gather indices (critical path) ----
            sums_i = sb.tile([128, 1], i32)
            with nc.allow_low_precision(reason="int32 sum of 0/1 mask is exact"):
                nc.vector.tensor_reduce(out=sums_i[:], in_=mask_t[:],
                                        axis=mybir.AxisListType.X,
                                        op=mybir.AluOpType.add)
            sums_f = sb.tile([128, 1], f32)
            nc.vector.tensor_copy(out=sums_f[:], in_=sums_i[:])
            # per-sample sums: group-of-8-partitions reduction via matmul,
            # plus a K=1 accumulation adding the per-row flat base j*S
            sep_psum = ps1.tile([BC, 1], f32)
            nc.tensor.matmul(out=sep_psum[:], lhsT=cpack[:1, C_ROWOFF:C_ROWOFF + BC],
                             rhs=cpack[:1, C_ONE1:C_ONE1 + 1],
                             start=True, stop=False)
            nc.tensor.matmul(out=sep_psum[:], lhsT=onesblk, rhs=sums_f[:],
                             start=False, stop=True)
            idx = sb.tile([BC, 1], i32)
            nc.vector.tensor_copy(out=idx[:], in_=sep_psum[:])

            # ---- gather the 16 sep rows straight from DRAM ----
            sep_rows = sb.tile([BC, H], f32)
            nc.gpsimd.indirect_dma_start(
                out=sep_rows[:], out_offset=None,
                in_=hid.ap(),
                in_offset=bass.IndirectOffsetOnAxis(ap=idx[:, :1], axis=0),
            )

            sepT = sb.tile([128, SC, BC], f32)
            sep_ps = ps2.tile([128, SC, BC], f32)
            for c in range(SC):
                nc.tensor.transpose(out=sep_ps[:, c, :],
                                    in_=sep_rows[:, c * 128:(c + 1) * 128],
                                    identity=eye)
            nc.vector.tensor_copy(out=sepT[:], in_=sep_ps[:])

            # ---- pred = [sep | cls] @ W.T + b: bias as a K=1 matmul from
            # cpack (zero new sems on PE), then 12 K-chunks ----
            pred = ps1.tile([BC, L], f32)
            nc.tensor.matmul(out=pred[:], lhsT=cpack[:1, C_ONER:C_ONER + BC],
                             rhs=cpack[:1, C_BIASR:C_BIASR + L],
                             start=True, stop=False)
            for c in range(SC):
                nc.tensor.matmul(out=pred[:], lhsT=clsT[:, c, :],
                                 rhs=wT[:, (SC + c) * L:(SC + c + 1) * L],
                                 start=False, stop=False)
            for c in range(SC):
                nc.tensor.matmul(out=pred[:], lhsT=sepT[:, c, :],
                                 rhs=wT[:, c * L:(c + 1) * L], start=False,
                                 stop=(c == SC - 1))

            out_sb = sb.tile([BC, L], f32)
            nc.vector.tensor_copy(out=out_sb[:], in_=pred[:])
            nc.sync.dma_start(out=outd.ap(), in_=out_sb[:])
    return nc


def _get_program():
    global _PROG
    if _PROG is None:
        _PROG = _build_program()
    return _PROG


def _make_in_maps(hidden_output, cls_outputs, input_mask, W, b):
    cpack = np.zeros((128, C_COLS), dtype=np.float32)
    # W[l, k] with k = c*128 + p  ->  wt[p, c*26 + l]
    cpack[:, C_WT:C_WT + KC * L] = np.ascontiguousarray(
        W.reshape(L, KC, 128).transpose(2, 1, 0)).reshape(128, KC * L)
    cpack[:, C_ONES:C_ONES + BC] = np.repeat(
        np.eye(BC, dtype=np.float32), 128 // BC, axis=0)
    cpack[:BC, C_EYE:C_EYE + BC] = np.eye(BC, dtype=np.float32)
    cpack[0, C_ROWOFF:C_ROWOFF + BC] = np.arange(BC, dtype=np.float32) * S
    cpack[0, C_ONE1] = 1.0
    cpack[0, C_BIASR:C_BIASR + L] = b
    cpack[0, C_ONER:C_ONER + BC] = 1.0

    in_maps = []
    for i in range(NCORES):
        s = slice(i * BC, (i + 1) * BC)
        in_maps.append({
            "hidden": np.ascontiguousarray(hidden_output[s]).reshape(BC * S, H),
            "cls": np.ascontiguousarray(cls_outputs[s]),
            "mask": np.ascontiguousarray(input_mask[s]).reshape(128, MQ),
            "cpack": cpack,
        })
    return in_maps


def kernel(hidden_output, cls_outputs, input_mask, W, b, **run_kwargs):
    nc = _get_program()
    in_maps = _make_in_maps(
        np.asarray(hidden_output, dtype=np.float32),
        np.asarray(cls_outputs, dtype=np.float32),
        np.asarray(input_mask, dtype=np.int32),
        np.asarray(W, dtype=np.float32),
        np.asarray(b, dtype=np.float32),
    )
    res = run_bass_kernel_spmd(nc, in_maps, core_ids=list(range(NCORES)),
                               **run_kwargs)
    out = np.concatenate([r["out"] for r in res.results], axis=0)
    if run_kwargs:
        return out, res
    return out
